# revision 20
# baseline (speedup 1.0000x reference)
"""Distributed multi-head attention kernel for 8 TRN2 NeuronCores.

Problem: B=2, S=2048, H=1024 (16 heads x 64), fp32 in/out.
Sharding: core c = 4*b + g handles batch b and head-group g (4 heads, 256
hidden cols). Wq/Wk/Wv column-sharded, Wo row-sharded; the cross-core
sum of output partials runs as a per-chunk collective over each 4-core
batch group.

v6: fully software-pipelined emission. Host pre-transposes x and
pre-arranges the weight tiles so every load is a contiguous 2KB-row
DMA. K/V/Q/output projections run as "filler" chain-pairs interleaved
into the attention inner loop (two accumulation chains with matmuls
alternating across two PSUM banks dual-pipeline at ~2x the rate of a
single chain). Each ctx pair is deferred one iteration so a filler pop
never sits between scores and the next exp. The exp() stream on ACT
(~136us) and the PE stream (~140us) are co-critical.

Dataflow per core (transpose-free attention, bf16 matmuls, fp32 PSUM):
  Q^T,K^T = (W^T x^T) in [j,t] layout; V = x^T-stationary @ Wv
  scores^T[k,q] = K^T.T@Q^T, two heads packed into PE row halves (K=64)
  Pt = exp(scores/8) (scores ~ N(0,1): exact softmax, no max pass)
  ctx^T[d,q] (+ sums row via ones column in V) = [V|1].T @ Pt
  normalize via K=1 broadcast matmul of sums + wide reciprocal (DVE)
  out partial[t,o] = ctx^T-stationary @ Wo -> bf16 -> per-chunk
  AllToAll + local DVE adds (or ReduceScatter with USE_A2A=False).
bq/bk applied on-device (DVE bias-add); bv/bo folded on host (exact:
out += bv@Wo + bo, since softmax rows sum to one).
"""

import sys

for p in ("/opt/trn_rl_repo",):
    if p not in sys.path:
        sys.path.insert(0, p)

from collections import deque
from contextlib import ExitStack

import ml_dtypes
import numpy as np

from concourse import bacc, mybir, tile
from concourse.bass import ds
from concourse.bass_utils import run_bass_kernel_spmd

F32 = mybir.dt.float32
BF16 = mybir.dt.bfloat16
AF = mybir.ActivationFunctionType

B, S, H = 2, 2048, 1024
NH, D = 16, 64
NCORES = 8
GROUPS = [[0, 1, 2, 3], [4, 5, 6, 7]]
JG = 256           # hidden cols per core (4 heads)
SO = S // 4        # 512 output rows per core after the collective

USE_A2A = False    # AllToAll needs >4-core groups; RS it is

_cache = {}


def _build():
    nc = bacc.Bacc("TRN2", target_bir_lowering=False, debug=False,
                   num_devices=NCORES)
    # weights arrive pre-arranged on host: w*[p, inner*s + j] = W[128s+p, j]
    xT_d = nc.dram_tensor("xT", [H, S], BF16, kind="ExternalInput")
    wq_d = nc.dram_tensor("wq", [128, 2048], BF16, kind="ExternalInput")
    wk_d = nc.dram_tensor("wk", [128, 2048], BF16, kind="ExternalInput")
    wv_d = nc.dram_tensor("wv", [128, 2048], BF16, kind="ExternalInput")
    wo_d = nc.dram_tensor("wo", [128, 2048], BF16, kind="ExternalInput")
    bq_d = nc.dram_tensor("bqc", [128, 2], F32, kind="ExternalInput")
    bk_d = nc.dram_tensor("bkc", [128, 2], F32, kind="ExternalInput")
    out_d = nc.dram_tensor("out", [SO, H], BF16, kind="ExternalOutput")

    def mm(ps, lhsT, rhs, start, stop, tile_position=None):
        nc.tensor.matmul(ps, lhsT, rhs, start=start, stop=stop,
                         tile_position=tile_position)

    with tile.TileContext(nc) as tc, ExitStack() as st:
        consts = st.enter_context(tc.tile_pool(name="consts", bufs=1))
        ones1 = consts.tile([1, 64], BF16)
        nc.vector.memset(ones1[:], 1.0)
        bq_sb = consts.tile([128, 2], F32)
        bk_sb = consts.tile([128, 2], F32)

        wpool = st.enter_context(tc.tile_pool(name="weights", bufs=1))
        w_sb = {}

        def load_w(wname, wd, eng):
            wt = wpool.tile([128, 2048], BF16, name=f"{wname}sb",
                            tag=f"{wname}sb")
            eng.dma_start(wt[:], wd.ap()[:, :])
            w_sb[wname] = wt

        # deadline order, on the fast hw-DGE queues: wq and wk first so
        # the first projection chains can start, then the xT chunks
        load_w("wq", wq_d, nc.sync)
        load_w("wk", wk_d, nc.scalar)
        nc.gpsimd.dma_start(bq_sb[:], bq_d[:, :])
        nc.gpsimd.dma_start(bk_sb[:], bk_d[:, :])
        load_w("wo", wo_d, nc.gpsimd)

        # x^T loads: 512-column chunks, chunk-major, split across the
        # sync and scalar hw DMA queues
        xTp = st.enter_context(tc.tile_pool(name="xT", bufs=1))
        xT = [xTp.tile([128, S], BF16, name=f"xT{s}", tag=f"xT{s}")
              for s in range(8)]
        for c in range(4):
            for s in range(8):
                eng = nc.sync if s < 4 else nc.scalar
                eng.dma_start(xT[s][:, ds(512 * c, 512)],
                              xT_d.ap()[ds(128 * s, 128), ds(512 * c, 512)])
            if c == 0:
                load_w("wv", wv_d, nc.sync)

        qkv = st.enter_context(tc.tile_pool(name="qkv", bufs=1))
        qT = [qkv.tile([128, S], BF16, name=f"qT{j}", tag=f"qT{j}")
              for j in range(2)]
        kT = [qkv.tile([128, S], BF16, name=f"kT{j}", tag=f"kT{j}")
              for j in range(2)]
        ctxT = [qkv.tile([128, S], BF16, name=f"cT{j}", tag=f"cT{j}")
                for j in range(2)]
        # V padded per head with a ones column: head h at cols 65h..65h+63
        v_sb = [qkv.tile([128, 260], BF16, name=f"v{i}", tag=f"v{i}")
                for i in range(16)]

        dram = st.enter_context(tc.tile_pool(name="dram", bufs=1, space="DRAM"))
        partial_c = [dram.tile([512, H], BF16, name=f"pc{i}", tag=f"pc{i}")
                     for i in range(4)]
        red_c = [dram.tile([512 if USE_A2A else 128, H], BF16,
                           name=f"rc{i}", tag=f"rc{i}")
                 for i in range(4)]

        with tc.tile_pool(name="scps", bufs=2, space="PSUM") as scps, \
             tc.tile_pool(name="ctxps", bufs=2, space="PSUM") as ctxps, \
             tc.tile_pool(name="fillps", bufs=2, space="PSUM") as fillps, \
             tc.tile_pool(name="psb", bufs=4) as psb, \
             tc.tile_pool(name="nrm", bufs=4) as nrm, \
             tc.tile_pool(name="osb", bufs=3) as osb, \
             tc.tile_pool(name="red", bufs=2) as red:

            # ---- filler chains (run interleaved with attention) ----
            # Consecutive matmuls into the SAME PSUM bank serialize
            # (~375ns/mm); alternating banks dual-pipeline (~213ns/mm).
            # So each filler emits TWO independent accumulation chains
            # with their matmuls interleaved across two banks.
            def qk_pair(which, c):
                w = w_sb["wq"] if which == "q" else w_sb["wk"]
                dstT = qT if which == "q" else kT
                bias = bq_sb if which == "q" else bk_sb

                def emit():
                    psA = fillps.tile([128, 512], F32, tag="fill")
                    psB = fillps.tile([128, 512], F32, tag="fill")
                    for s in range(8):
                        mm(psA[:], w[:, ds(256 * s, 128)],
                           xT[s][:, ds(512 * c, 512)], s == 0, s == 7)
                        mm(psB[:], w[:, ds(256 * s + 128, 128)],
                           xT[s][:, ds(512 * c, 512)], s == 0, s == 7)
                    nc.vector.tensor_scalar_add(
                        dstT[0][:, ds(512 * c, 512)], psA[:], bias[:, ds(0, 1)])
                    nc.vector.tensor_scalar_add(
                        dstT[1][:, ds(512 * c, 512)], psB[:], bias[:, ds(1, 1)])
                return emit

            def v_pair(tv0, tv1):
                def emit():
                    psA = fillps.tile([128, 512], F32, tag="fill")
                    psB = fillps.tile([128, 512], F32, tag="fill")
                    for s in range(8):
                        mm(psA[:, 0:256], xT[s][:, ds(128 * tv0, 128)],
                           w_sb["wv"][:, ds(256 * s, 256)], s == 0, s == 7)
                        if tv1 is not None:
                            mm(psB[:, 0:256], xT[s][:, ds(128 * tv1, 128)],
                               w_sb["wv"][:, ds(256 * s, 256)], s == 0, s == 7)
                    for tv, ps in ((tv0, psA), (tv1, psB)):
                        if tv is None:
                            continue
                        nc.vector.memset(v_sb[tv][:], 1.0)
                        nc.vector.tensor_copy(
                            v_sb[tv][:].rearrange("p (h c) -> p h c", c=65)[:, :, 0:64],
                            ps[:, 0:256].rearrange("p (h c) -> p h c", c=64))
                return emit

            stages = {}

            def o_pair(tq, tl):
                def emit():
                    tt = 4 * tq + tl
                    if tl == 0:
                        stages[tq] = osb.tile([128, 4096], BF16,
                                              name=f"ot{tq}", tag="ot")
                    stage = stages[tq]
                    psA = fillps.tile([128, 512], F32, tag="fill")
                    psB = fillps.tile([128, 512], F32, tag="fill")
                    for idx, js in enumerate((1, 0)):
                        mm(psA[:], ctxT[js][:, ds(128 * tt, 128)],
                           w_sb["wo"][:, ds(1024 * js, 512)],
                           idx == 0, idx == 1)
                        mm(psB[:], ctxT[js][:, ds(128 * tt, 128)],
                           w_sb["wo"][:, ds(1024 * js + 512, 512)],
                           idx == 0, idx == 1)
                    nc.vector.tensor_copy(stage[:, ds(1024 * tl, 512)], psA[:])
                    nc.vector.tensor_copy(
                        stage[:, ds(1024 * tl + 512, 512)], psB[:])
                    # one 256KB DMA per token-block: four parallel rings
                    nc.sync.dma_start(partial_c[tq][ds(128 * tl, 128), :],
                                      stage[:, ds(1024 * tl, 1024)])
                return emit

            def red_op(tq):
                def emit():
                    if USE_A2A:
                        nc.gpsimd.collective_compute(
                            "AllToAll", mybir.AluOpType.bypass,
                            replica_groups=GROUPS,
                            ins=[partial_c[tq].opt()], outs=[red_c[tq].opt()])
                        # gather the 4 partial pieces and sum on DVE
                        gat = red.tile([128, 4096], BF16, name=f"ga{tq}",
                                       tag="gat")
                        for pc in range(4):
                            nc.sync.dma_start(
                                gat[:, ds(1024 * pc, 1024)],
                                red_c[tq][ds(128 * pc, 128), :])
                        g = gat[:].rearrange("p (pc o) -> p pc o", o=1024)
                        s01 = red.tile([128, 1024], BF16, name=f"s0{tq}",
                                       tag="s01")
                        nc.vector.tensor_add(s01[:], g[:, 0], g[:, 1])
                        s23 = red.tile([128, 1024], BF16, name=f"s2{tq}",
                                       tag="s23")
                        nc.vector.tensor_add(s23[:], g[:, 2], g[:, 3])
                        stot = red.tile([128, 1024], BF16, name=f"st{tq}",
                                        tag="stot")
                        nc.vector.tensor_add(stot[:], s01[:], s23[:])
                        for half in range(2):
                            nc.sync.dma_start(
                                out_d[ds(128 * tq, 128), ds(512 * half, 512)],
                                stot[:, ds(512 * half, 512)])
                    else:
                        nc.gpsimd.collective_compute(
                            "ReduceScatter", mybir.AluOpType.add,
                            replica_groups=GROUPS,
                            ins=[partial_c[tq].opt()], outs=[red_c[tq].opt()])
                        nc.gpsimd.dma_start(out_d[ds(128 * tq, 128), :],
                                            red_c[tq][:])
                return emit

            # warm up the PE while the xT/weight DMAs stream: a cold
            # tensor engine runs at less than half rate for its first
            # ~3us, so burn that ramp on junk matmuls during the loads
            junk = consts.tile([128, 512], BF16, name="junk", tag="junk")
            nc.vector.memset(junk[:], 0.0)
            junko = consts.tile([128, 8], F32, name="junko", tag="junko")
            wps = [scps.tile([128, 1024], F32, name=f"wp{i}", tag="sp")
                   for i in range(2)]
            for i in range(14):
                mm(wps[i % 2][:, 0:512], junk[:, 0:128], junk[:], True, True)
            for i in range(2):
                nc.vector.tensor_copy(junko[:, ds(4 * i, 4)], wps[i][:, 0:4])

            # pre-loop: Q(tq0), K(c0), V(tv0) emitted directly
            qk_pair("q", 0)()
            qk_pair("k", 0)()
            v_pair(0, None)()

            fillers = deque()
            fillers.append(v_pair(1, 2))
            fillers.append(qk_pair("k", 1))
            fillers.append(v_pair(3, 4))
            fillers.append(qk_pair("k", 2))
            fillers.append(v_pair(5, 6))
            fillers.append(qk_pair("k", 3))
            fillers.append(v_pair(7, 8))
            fillers.append(v_pair(9, 10))
            fillers.append(v_pair(11, 12))
            fillers.append(v_pair(13, 14))
            fillers.append(v_pair(15, None))
            fillers.append(qk_pair("q", 1))

            def emit_norm(pending):
                # previous loop's normalize: broadcast raw sums via K=1
                # matmuls, wide reciprocal + multiply on DVE
                ptq, php, pcA, pcB, sms = pending
                bcs = []
                for sm16 in sms:
                    bc = fillps.tile([128, 512], F32, tag="fill")
                    mm(bc[0:64, :], ones1[:], sm16[:], True, True)
                    bcs.append(bc)
                for h, cps, bc in ((2 * php, pcA, bcs[0]),
                                   (2 * php + 1, pcB, bcs[1])):
                    rbc = nrm.tile([64, 512], F32, tag="rbc")
                    nc.vector.reciprocal_approx_fast(rbc[:], bc[0:64, :])
                    nc.vector.tensor_mul(
                        ctxT[php][ds(64 * (h % 2), 64), ds(512 * ptq, 512)],
                        cps[0:64, :], rbc[:])

            pending = None
            for li, (tq, hp) in enumerate(
                    (t, h) for t in range(4) for h in (0, 1)):
                if tq >= 1 and hp == 0 and tq < 3:
                    fillers.append(qk_pair("q", tq + 1))
                cA = ctxps.tile([65, 512], F32, tag="cps")
                cB = ctxps.tile([65, 512], F32, tag="cps")

                def ctx_pair(kt, pt, hp=hp, cA=cA, cB=cB):
                    mm(cA[:], v_sb[kt][:, ds(65 * (2 * hp), 65)],
                       pt[:, 0:512], kt == 0, kt == 15)
                    mm(cB[:], v_sb[kt][:, ds(65 * (2 * hp + 1), 65)],
                       pt[:, 512:1024], kt == 0, kt == 15)

                # ctx pairs run one iteration deferred: a filler pop never
                # separates scores(kt+1) from exp(kt+1), so the ACT stream
                # stays gapless across pops and loop transitions
                pend_ctx = None
                for kt in range(16):
                    sp = scps.tile([128, 1024], F32, tag="sp")
                    mm(sp[:, 0:512],
                       kT[hp][0:64, ds(128 * kt, 128)],
                       qT[hp][0:64, ds(512 * tq, 512)],
                       True, True, tile_position=(0, 0))
                    mm(sp[:, 512:1024],
                       kT[hp][64:128, ds(128 * kt, 128)],
                       qT[hp][64:128, ds(512 * tq, 512)],
                       True, True, tile_position=(64, 0))
                    pt = psb.tile([128, 1024], BF16, tag="pt")
                    nc.scalar.activation(pt[:], sp[:], AF.Exp, scale=0.125)
                    if kt == 1 and pending is not None:
                        emit_norm(pending)
                        pending = None
                    if kt >= 2 and fillers:
                        fillers.popleft()()
                    if pend_ctx is not None:
                        ctx_pair(*pend_ctx)
                    pend_ctx = (kt, pt)
                ctx_pair(*pend_ctx)
                # cast both heads' sums rows now (DVE); the bc matmuls and
                # multiplies run early in the next loop
                sms = []
                for cps in (cA, cB):
                    sm16 = nrm.tile([1, 512], BF16, tag="sm")
                    nc.vector.tensor_copy(sm16[:], cps[ds(64, 1), :])
                    sms.append(sm16)
                pending = (tq, hp, cA, cB, sms)
                if hp == 1:
                    for tl in range(4):
                        fillers.append(o_pair(tq, tl))
                    fillers.append(red_op(tq))

            emit_norm(pending)
            while fillers:
                fillers.popleft()()

    nc.compile()
    return nc


def _get_nc():
    if "nc" not in _cache:
        _cache["nc"] = _build()
    return _cache["nc"]


def _arr_w(Wslice):
    # [1024, 256] -> [128, 2048] with w[p, 256s+j] = W[128s+p, j]
    return np.ascontiguousarray(
        Wslice.reshape(8, 128, 256).transpose(1, 0, 2).reshape(128, 2048))


def _in_maps(x, Wq, bq, Wk, bk, Wv, bv, Wo, bo):
    bf = ml_dtypes.bfloat16
    maps = []
    for c in range(NCORES):
        b, g = c // 4, c % 4
        j0 = JG * g
        wo_slice = Wo[j0:j0 + JG, :]  # [256, 1024]
        wo_arr = np.ascontiguousarray(
            wo_slice.reshape(2, 128, 1024).transpose(1, 0, 2).reshape(128, 2048))
        maps.append({
            "xT": np.ascontiguousarray(x[b].T).astype(bf),
            "wq": _arr_w(Wq[:, j0:j0 + JG]).astype(bf),
            "wk": _arr_w(Wk[:, j0:j0 + JG]).astype(bf),
            "wv": _arr_w(Wv[:, j0:j0 + JG]).astype(bf),
            "wo": wo_arr.astype(bf),
            "bqc": np.ascontiguousarray(bq[j0:j0 + JG].reshape(2, 128).T),
            "bkc": np.ascontiguousarray(bk[j0:j0 + JG].reshape(2, 128).T),
        })
    return maps


def kernel(x, Wq, bq, Wk, bk, Wv, bv, Wo, bo, _trace=False):
    x, Wq, bq, Wk, bk, Wv, bv, Wo, bo = (
        np.asarray(a, dtype=np.float32)
        for a in (x, Wq, bq, Wk, bk, Wv, bv, Wo, bo))
    nc = _get_nc()
    res = run_bass_kernel_spmd(nc, _in_maps(x, Wq, bq, Wk, bk, Wv, bv, Wo, bo),
                               core_ids=list(range(NCORES)), trace=_trace)
    out = np.empty((B, S, H), np.float32)
    for c in range(NCORES):
        b, g = c // 4, c % 4
        oc = np.asarray(res.results[c]["out"], dtype=np.float32)
        for tq in range(4):
            out[b, 512 * tq + 128 * g:512 * tq + 128 * (g + 1), :] = \
                oc[128 * tq:128 * (tq + 1)]
    out += bv @ Wo + bo  # exact: softmax rows sum to 1
    if _trace:
        return out, res
    return out


# revision 23
# speedup vs baseline: 1.0349x; 1.0349x over previous
"""Distributed multi-head attention kernel for 8 TRN2 NeuronCores.

Problem: B=2, S=2048, H=1024 (16 heads x 64), fp32 in/out.
Sharding: core c = 4*b + g handles batch b and head-group g (4 heads, 256
hidden cols). Wq/Wk/Wv column-sharded, Wo row-sharded; the cross-core
sum of output partials runs as a per-chunk collective over each 4-core
batch group.

v6: fully software-pipelined emission. Host pre-transposes x and
pre-arranges the weight tiles so every load is a contiguous 2KB-row
DMA. K/V/Q/output projections run as "filler" chain-pairs interleaved
into the attention inner loop (two accumulation chains with matmuls
alternating across two PSUM banks dual-pipeline at ~2x the rate of a
single chain). Each ctx pair is deferred one iteration so a filler pop
never sits between scores and the next exp. The exp() stream on ACT
(~136us) and the PE stream (~140us) are co-critical.

Dataflow per core (transpose-free attention, bf16 matmuls, fp32 PSUM):
  Q^T,K^T = (W^T x^T) in [j,t] layout; V = x^T-stationary @ Wv
  scores^T[k,q] = K^T.T@Q^T, two heads packed into PE row halves (K=64)
  Pt = exp(scores/8) (scores ~ N(0,1): exact softmax, no max pass)
  ctx^T[d,q] (+ sums row via ones column in V) = [V|1].T @ Pt
  normalize via K=1 broadcast matmul of sums + wide reciprocal (DVE)
  out partial[t,o] = ctx^T-stationary @ Wo -> bf16 -> per-chunk
  AllToAll + local DVE adds (or ReduceScatter with USE_A2A=False).
bq/bk applied on-device (DVE bias-add); bv/bo folded on host (exact:
out += bv@Wo + bo, since softmax rows sum to one).
"""

import sys

for p in ("/opt/trn_rl_repo",):
    if p not in sys.path:
        sys.path.insert(0, p)

from collections import deque
from contextlib import ExitStack

import ml_dtypes
import numpy as np

from concourse import bacc, mybir, tile
from concourse.bass import ds
from concourse.bass_utils import run_bass_kernel_spmd

F32 = mybir.dt.float32
BF16 = mybir.dt.bfloat16
AF = mybir.ActivationFunctionType

B, S, H = 2, 2048, 1024
NH, D = 16, 64
NCORES = 8
GROUPS = [[0, 1, 2, 3], [4, 5, 6, 7]]
JG = 256           # hidden cols per core (4 heads)
SO = S // 4        # 512 output rows per core after the collective

USE_A2A = False    # AllToAll needs >4-core groups; RS it is

_cache = {}


def _build():
    nc = bacc.Bacc("TRN2", target_bir_lowering=False, debug=False,
                   num_devices=NCORES)
    # weights arrive pre-arranged on host: w*[p, inner*s + j] = W[128s+p, j]
    xT_d = nc.dram_tensor("xT", [H, S], BF16, kind="ExternalInput")
    wq_d = nc.dram_tensor("wq", [128, 2048], BF16, kind="ExternalInput")
    wk_d = nc.dram_tensor("wk", [128, 2048], BF16, kind="ExternalInput")
    wv_d = nc.dram_tensor("wv", [128, 2048], BF16, kind="ExternalInput")
    wo_d = nc.dram_tensor("wo", [128, 2048], BF16, kind="ExternalInput")
    bq_d = nc.dram_tensor("bqc", [128, 2], F32, kind="ExternalInput")
    bk_d = nc.dram_tensor("bkc", [128, 2], F32, kind="ExternalInput")
    out_d = nc.dram_tensor("out", [SO, H], BF16, kind="ExternalOutput")

    def mm(ps, lhsT, rhs, start, stop, tile_position=None):
        nc.tensor.matmul(ps, lhsT, rhs, start=start, stop=stop,
                         tile_position=tile_position)

    with tile.TileContext(nc) as tc, ExitStack() as st:
        consts = st.enter_context(tc.tile_pool(name="consts", bufs=1))
        ones1 = consts.tile([1, 64], BF16)
        nc.vector.memset(ones1[:], 1.0)
        bq_sb = consts.tile([128, 2], F32)
        bk_sb = consts.tile([128, 2], F32)

        wpool = st.enter_context(tc.tile_pool(name="weights", bufs=1))
        w_sb = {}

        def load_w(wname, wd, eng):
            wt = wpool.tile([128, 2048], BF16, name=f"{wname}sb",
                            tag=f"{wname}sb")
            eng.dma_start(wt[:], wd.ap()[:, :])
            w_sb[wname] = wt

        # deadline order, on the fast hw-DGE queues: wq and wk first so
        # the first projection chains can start, then the xT chunks
        load_w("wq", wq_d, nc.sync)
        load_w("wk", wk_d, nc.scalar)
        nc.gpsimd.dma_start(bq_sb[:], bq_d[:, :])
        nc.gpsimd.dma_start(bk_sb[:], bk_d[:, :])
        load_w("wo", wo_d, nc.gpsimd)

        # x^T loads: 512-column chunks, chunk-major, split across the
        # sync and scalar hw DMA queues
        xTp = st.enter_context(tc.tile_pool(name="xT", bufs=1))
        xT = [xTp.tile([128, S], BF16, name=f"xT{s}", tag=f"xT{s}")
              for s in range(8)]
        for c in range(4):
            for s in range(8):
                eng = nc.sync if s < 4 else nc.scalar
                eng.dma_start(xT[s][:, ds(512 * c, 512)],
                              xT_d.ap()[ds(128 * s, 128), ds(512 * c, 512)])
            if c == 0:
                load_w("wv", wv_d, nc.sync)

        qkv = st.enter_context(tc.tile_pool(name="qkv", bufs=1))
        qT = [qkv.tile([128, S], BF16, name=f"qT{j}", tag=f"qT{j}")
              for j in range(2)]
        kT = [qkv.tile([128, S], BF16, name=f"kT{j}", tag=f"kT{j}")
              for j in range(2)]
        ctxT = [qkv.tile([128, S], BF16, name=f"cT{j}", tag=f"cT{j}")
                for j in range(2)]
        # V padded per head with a ones column: head h at cols 65h..65h+63
        v_sb = [qkv.tile([128, 260], BF16, name=f"v{i}", tag=f"v{i}")
                for i in range(16)]

        dram = st.enter_context(tc.tile_pool(name="dram", bufs=1, space="DRAM"))
        partial_c = [dram.tile([512, H], BF16, name=f"pc{i}", tag=f"pc{i}")
                     for i in range(4)]
        red_c = [dram.tile([512 if USE_A2A else 128, H], BF16,
                           name=f"rc{i}", tag=f"rc{i}")
                 for i in range(4)]

        with tc.tile_pool(name="scps", bufs=2, space="PSUM") as scps, \
             tc.tile_pool(name="ctxps", bufs=2, space="PSUM") as ctxps, \
             tc.tile_pool(name="fillps", bufs=2, space="PSUM") as fillps, \
             tc.tile_pool(name="psb", bufs=4) as psb, \
             tc.tile_pool(name="nrm", bufs=4) as nrm, \
             tc.tile_pool(name="osb", bufs=3) as osb, \
             tc.tile_pool(name="red", bufs=2) as red:

            # ---- filler chains (run interleaved with attention) ----
            # Consecutive matmuls into the SAME PSUM bank serialize
            # (~375ns/mm); alternating banks dual-pipeline (~213ns/mm).
            # So each filler emits TWO independent accumulation chains
            # with their matmuls interleaved across two banks.
            def qk_pair(which, c):
                w = w_sb["wq"] if which == "q" else w_sb["wk"]
                dstT = qT if which == "q" else kT
                bias = bq_sb if which == "q" else bk_sb

                def emit():
                    psA = fillps.tile([128, 512], F32, tag="fill")
                    psB = fillps.tile([128, 512], F32, tag="fill")
                    for s in range(8):
                        mm(psA[:], w[:, ds(256 * s, 128)],
                           xT[s][:, ds(512 * c, 512)], s == 0, s == 7)
                        mm(psB[:], w[:, ds(256 * s + 128, 128)],
                           xT[s][:, ds(512 * c, 512)], s == 0, s == 7)
                    nc.vector.tensor_scalar_add(
                        dstT[0][:, ds(512 * c, 512)], psA[:], bias[:, ds(0, 1)])
                    nc.vector.tensor_scalar_add(
                        dstT[1][:, ds(512 * c, 512)], psB[:], bias[:, ds(1, 1)])
                return emit

            def v_pair(tv0, tv1):
                def emit():
                    psA = fillps.tile([128, 512], F32, tag="fill")
                    psB = fillps.tile([128, 512], F32, tag="fill")
                    for s in range(8):
                        mm(psA[:, 0:256], xT[s][:, ds(128 * tv0, 128)],
                           w_sb["wv"][:, ds(256 * s, 256)], s == 0, s == 7)
                        if tv1 is not None:
                            mm(psB[:, 0:256], xT[s][:, ds(128 * tv1, 128)],
                               w_sb["wv"][:, ds(256 * s, 256)], s == 0, s == 7)
                    for tv, ps in ((tv0, psA), (tv1, psB)):
                        if tv is None:
                            continue
                        nc.vector.memset(v_sb[tv][:], 1.0)
                        nc.vector.tensor_copy(
                            v_sb[tv][:].rearrange("p (h c) -> p h c", c=65)[:, :, 0:64],
                            ps[:, 0:256].rearrange("p (h c) -> p h c", c=64))
                return emit

            stages = {}

            def o_pair(tq, tl):
                def emit():
                    tt = 4 * tq + tl
                    if tl == 0:
                        stages[tq] = osb.tile([128, 4096], BF16,
                                              name=f"ot{tq}", tag="ot")
                    stage = stages[tq]
                    psA = fillps.tile([128, 512], F32, tag="fill")
                    psB = fillps.tile([128, 512], F32, tag="fill")
                    for idx, js in enumerate((1, 0)):
                        mm(psA[:], ctxT[js][:, ds(128 * tt, 128)],
                           w_sb["wo"][:, ds(1024 * js, 512)],
                           idx == 0, idx == 1)
                        mm(psB[:], ctxT[js][:, ds(128 * tt, 128)],
                           w_sb["wo"][:, ds(1024 * js + 512, 512)],
                           idx == 0, idx == 1)
                    nc.vector.tensor_copy(stage[:, ds(1024 * tl, 512)], psA[:])
                    nc.vector.tensor_copy(
                        stage[:, ds(1024 * tl + 512, 512)], psB[:])
                    # one 256KB DMA per token-block: four parallel rings
                    nc.sync.dma_start(partial_c[tq][ds(128 * tl, 128), :],
                                      stage[:, ds(1024 * tl, 1024)])
                return emit

            def red_op(tq):
                def emit():
                    if USE_A2A:
                        nc.gpsimd.collective_compute(
                            "AllToAll", mybir.AluOpType.bypass,
                            replica_groups=GROUPS,
                            ins=[partial_c[tq].opt()], outs=[red_c[tq].opt()])
                        # gather the 4 partial pieces and sum on DVE
                        gat = red.tile([128, 4096], BF16, name=f"ga{tq}",
                                       tag="gat")
                        for pc in range(4):
                            nc.sync.dma_start(
                                gat[:, ds(1024 * pc, 1024)],
                                red_c[tq][ds(128 * pc, 128), :])
                        g = gat[:].rearrange("p (pc o) -> p pc o", o=1024)
                        s01 = red.tile([128, 1024], BF16, name=f"s0{tq}",
                                       tag="s01")
                        nc.vector.tensor_add(s01[:], g[:, 0], g[:, 1])
                        s23 = red.tile([128, 1024], BF16, name=f"s2{tq}",
                                       tag="s23")
                        nc.vector.tensor_add(s23[:], g[:, 2], g[:, 3])
                        stot = red.tile([128, 1024], BF16, name=f"st{tq}",
                                        tag="stot")
                        nc.vector.tensor_add(stot[:], s01[:], s23[:])
                        for half in range(2):
                            nc.sync.dma_start(
                                out_d[ds(128 * tq, 128), ds(512 * half, 512)],
                                stot[:, ds(512 * half, 512)])
                    else:
                        nc.gpsimd.collective_compute(
                            "ReduceScatter", mybir.AluOpType.add,
                            replica_groups=GROUPS,
                            ins=[partial_c[tq].opt()], outs=[red_c[tq].opt()])
                        nc.gpsimd.dma_start(out_d[ds(128 * tq, 128), :],
                                            red_c[tq][:])
                return emit

            # warm up the PE while the xT/weight DMAs stream: a cold
            # tensor engine runs at less than half rate for its first
            # ~3us, so burn that ramp on junk matmuls during the loads
            junk = consts.tile([128, 512], BF16, name="junk", tag="junk")
            nc.vector.memset(junk[:], 0.0)
            junko = consts.tile([128, 8], F32, name="junko", tag="junko")
            wps = [scps.tile([128, 1024], F32, name=f"wp{i}", tag="sp")
                   for i in range(2)]
            for i in range(14):
                mm(wps[i % 2][:, 0:512], junk[:, 0:128], junk[:], True, True)
            for i in range(2):
                nc.vector.tensor_copy(junko[:, ds(4 * i, 4)], wps[i][:, 0:4])

            # pre-loop: Q(tq0), K(c0), V(tv0) emitted directly
            qk_pair("q", 0)()
            qk_pair("k", 0)()
            v_pair(0, None)()

            fillers = deque()
            fillers.append(v_pair(1, 2))
            fillers.append(qk_pair("k", 1))
            fillers.append(v_pair(3, 4))
            fillers.append(qk_pair("k", 2))
            fillers.append(v_pair(5, 6))
            fillers.append(qk_pair("k", 3))
            fillers.append(v_pair(7, 8))
            fillers.append(v_pair(9, 10))
            fillers.append(v_pair(11, 12))
            fillers.append(v_pair(13, 14))
            fillers.append(v_pair(15, None))

            def emit_norm(pending):
                # previous loop's normalize: broadcast raw sums via K=1
                # matmuls, wide reciprocal + multiply on DVE
                ptq, php, pcA, pcB, sms = pending
                bcs = []
                for sm16 in sms:
                    bc = fillps.tile([128, 512], F32, tag="fill")
                    mm(bc[0:64, :], ones1[:], sm16[:], True, True)
                    bcs.append(bc)
                for h, cps, bc in ((2 * php, pcA, bcs[0]),
                                   (2 * php + 1, pcB, bcs[1])):
                    rbc = nrm.tile([64, 512], F32, tag="rbc")
                    nc.vector.reciprocal_approx_fast(rbc[:], bc[0:64, :])
                    nc.vector.tensor_mul(
                        ctxT[php][ds(64 * (h % 2), 64), ds(512 * ptq, 512)],
                        cps[0:64, :], rbc[:])

            pending = None
            for li, (tq, hp) in enumerate(
                    (t, h) for t in range(4) for h in (0, 1)):
                cA = ctxps.tile([65, 512], F32, tag="cps")
                cB = ctxps.tile([65, 512], F32, tag="cps")

                def ctx_pair(kt, pt, hp=hp, cA=cA, cB=cB):
                    mm(cA[:], v_sb[kt][:, ds(65 * (2 * hp), 65)],
                       pt[:, 0:512], kt == 0, kt == 15)
                    mm(cB[:], v_sb[kt][:, ds(65 * (2 * hp + 1), 65)],
                       pt[:, 512:1024], kt == 0, kt == 15)

                # ctx pairs run one iteration deferred: a filler pop never
                # separates scores(kt+1) from exp(kt+1), so the ACT stream
                # stays gapless across pops and loop transitions
                pend_ctx = None
                for kt in range(16):
                    sp = scps.tile([128, 1024], F32, tag="sp")
                    mm(sp[:, 0:512],
                       kT[hp][0:64, ds(128 * kt, 128)],
                       qT[hp][0:64, ds(512 * tq, 512)],
                       True, True, tile_position=(0, 0))
                    mm(sp[:, 512:1024],
                       kT[hp][64:128, ds(128 * kt, 128)],
                       qT[hp][64:128, ds(512 * tq, 512)],
                       True, True, tile_position=(64, 0))
                    pt = psb.tile([128, 1024], BF16, tag="pt")
                    nc.scalar.activation(pt[:], sp[:], AF.Exp, scale=0.125)
                    if kt == 1 and pending is not None:
                        emit_norm(pending)
                        pending = None
                    if kt >= 2 and fillers:
                        fillers.popleft()()
                    if pend_ctx is not None:
                        ctx_pair(*pend_ctx)
                    pend_ctx = (kt, pt)
                ctx_pair(*pend_ctx)
                # cast both heads' sums rows now (DVE); the bc matmuls and
                # multiplies run early in the next loop
                sms = []
                for cps in (cA, cB):
                    sm16 = nrm.tile([1, 512], BF16, tag="sm")
                    nc.vector.tensor_copy(sm16[:], cps[ds(64, 1), :])
                    sms.append(sm16)
                pending = (tq, hp, cA, cB, sms)
                if hp == 0 and tq < 3:
                    # Q projection for the next chunk pops during the
                    # otherwise-filler-free (tq, hp=1) loop, keeping the
                    # lumpy 3.4us chain pairs out of the o_pair loops
                    fillers.append(qk_pair("q", tq + 1))
                if hp == 1:
                    for tl in range(4):
                        fillers.append(o_pair(tq, tl))
                    fillers.append(red_op(tq))

            emit_norm(pending)
            while fillers:
                fillers.popleft()()

    nc.compile()
    return nc


def _get_nc():
    if "nc" not in _cache:
        _cache["nc"] = _build()
    return _cache["nc"]


def _arr_w(Wslice):
    # [1024, 256] -> [128, 2048] with w[p, 256s+j] = W[128s+p, j]
    return np.ascontiguousarray(
        Wslice.reshape(8, 128, 256).transpose(1, 0, 2).reshape(128, 2048))


def _in_maps(x, Wq, bq, Wk, bk, Wv, bv, Wo, bo):
    bf = ml_dtypes.bfloat16
    maps = []
    for c in range(NCORES):
        b, g = c // 4, c % 4
        j0 = JG * g
        wo_slice = Wo[j0:j0 + JG, :]  # [256, 1024]
        wo_arr = np.ascontiguousarray(
            wo_slice.reshape(2, 128, 1024).transpose(1, 0, 2).reshape(128, 2048))
        maps.append({
            "xT": np.ascontiguousarray(x[b].T).astype(bf),
            "wq": _arr_w(Wq[:, j0:j0 + JG]).astype(bf),
            "wk": _arr_w(Wk[:, j0:j0 + JG]).astype(bf),
            "wv": _arr_w(Wv[:, j0:j0 + JG]).astype(bf),
            "wo": wo_arr.astype(bf),
            "bqc": np.ascontiguousarray(bq[j0:j0 + JG].reshape(2, 128).T),
            "bkc": np.ascontiguousarray(bk[j0:j0 + JG].reshape(2, 128).T),
        })
    return maps


def kernel(x, Wq, bq, Wk, bk, Wv, bv, Wo, bo, _trace=False):
    x, Wq, bq, Wk, bk, Wv, bv, Wo, bo = (
        np.asarray(a, dtype=np.float32)
        for a in (x, Wq, bq, Wk, bk, Wv, bv, Wo, bo))
    nc = _get_nc()
    res = run_bass_kernel_spmd(nc, _in_maps(x, Wq, bq, Wk, bk, Wv, bv, Wo, bo),
                               core_ids=list(range(NCORES)), trace=_trace)
    out = np.empty((B, S, H), np.float32)
    for c in range(NCORES):
        b, g = c // 4, c % 4
        oc = np.asarray(res.results[c]["out"], dtype=np.float32)
        for tq in range(4):
            out[b, 512 * tq + 128 * g:512 * tq + 128 * (g + 1), :] = \
                oc[128 * tq:128 * (tq + 1)]
    out += bv @ Wo + bo  # exact: softmax rows sum to 1
    if _trace:
        return out, res
    return out


# revision 26
# speedup vs baseline: 1.0649x; 1.0289x over previous
"""Distributed multi-head attention kernel for 8 TRN2 NeuronCores.

Problem: B=2, S=2048, H=1024 (16 heads x 64), fp32 in/out.
Sharding: core c = 4*b + g handles batch b and head-group g (4 heads, 256
hidden cols). Wq/Wk/Wv column-sharded, Wo row-sharded; the cross-core
sum of output partials runs as a per-chunk collective over each 4-core
batch group.

v6: fully software-pipelined emission. Host pre-transposes x and
pre-arranges the weight tiles so every load is a contiguous 2KB-row
DMA. K/V/Q/output projections run as "filler" chain-pairs interleaved
into the attention inner loop (two accumulation chains with matmuls
alternating across two PSUM banks dual-pipeline at ~2x the rate of a
single chain). Each ctx pair is deferred one iteration so a filler pop
never sits between scores and the next exp. The exp() stream on ACT
(~136us) and the PE stream (~140us) are co-critical.

Dataflow per core (transpose-free attention, bf16 matmuls, fp32 PSUM):
  Q^T,K^T = (W^T x^T) in [j,t] layout; V = x^T-stationary @ Wv
  scores^T[k,q] = K^T.T@Q^T, two heads packed into PE row halves (K=64)
  Pt = exp(scores/8) (scores ~ N(0,1): exact softmax, no max pass)
  ctx^T[d,q] (+ sums row via ones column in V) = [V|1].T @ Pt
  normalize via K=1 broadcast matmul of sums + wide reciprocal (DVE)
  out partial[t,o] = ctx^T-stationary @ Wo -> bf16 -> per-chunk
  AllToAll + local DVE adds (or ReduceScatter with USE_A2A=False).
bq/bk applied on-device (DVE bias-add); bv/bo folded on host (exact:
out += bv@Wo + bo, since softmax rows sum to one).
"""

import sys

for p in ("/opt/trn_rl_repo",):
    if p not in sys.path:
        sys.path.insert(0, p)

from collections import deque
from contextlib import ExitStack

import ml_dtypes
import numpy as np

from concourse import bacc, mybir, tile
from concourse.bass import ds
from concourse.bass_utils import run_bass_kernel_spmd

F32 = mybir.dt.float32
BF16 = mybir.dt.bfloat16
AF = mybir.ActivationFunctionType

B, S, H = 2, 2048, 1024
NH, D = 16, 64
NCORES = 8
GROUPS = [[0, 1, 2, 3], [4, 5, 6, 7]]
JG = 256           # hidden cols per core (4 heads)
SO = S // 4        # 512 output rows per core after the collective

USE_A2A = False    # AllToAll needs >4-core groups; RS it is

_cache = {}


def _build():
    nc = bacc.Bacc("TRN2", target_bir_lowering=False, debug=False,
                   num_devices=NCORES)
    # weights arrive pre-arranged on host: w*[p, inner*s + j] = W[128s+p, j]
    xT_d = nc.dram_tensor("xT", [H, S], BF16, kind="ExternalInput")
    wq_d = nc.dram_tensor("wq", [128, 2048], BF16, kind="ExternalInput")
    wk_d = nc.dram_tensor("wk", [128, 2048], BF16, kind="ExternalInput")
    wv_d = nc.dram_tensor("wv", [128, 2048], BF16, kind="ExternalInput")
    wo_d = nc.dram_tensor("wo", [128, 2048], BF16, kind="ExternalInput")
    bq_d = nc.dram_tensor("bqc", [128, 2], F32, kind="ExternalInput")
    bk_d = nc.dram_tensor("bkc", [128, 2], F32, kind="ExternalInput")
    out_d = nc.dram_tensor("out", [SO, H], BF16, kind="ExternalOutput")

    def mm(ps, lhsT, rhs, start, stop, tile_position=None):
        nc.tensor.matmul(ps, lhsT, rhs, start=start, stop=stop,
                         tile_position=tile_position)

    with tile.TileContext(nc) as tc, ExitStack() as st:
        consts = st.enter_context(tc.tile_pool(name="consts", bufs=1))
        ones1 = consts.tile([1, 64], BF16)
        nc.vector.memset(ones1[:], 1.0)
        bq_sb = consts.tile([128, 2], F32)
        bk_sb = consts.tile([128, 2], F32)

        wpool = st.enter_context(tc.tile_pool(name="weights", bufs=1))
        w_sb = {}

        def load_w(wname, wd, eng):
            wt = wpool.tile([128, 2048], BF16, name=f"{wname}sb",
                            tag=f"{wname}sb")
            eng.dma_start(wt[:], wd.ap()[:, :])
            w_sb[wname] = wt

        # deadline order, on the fast hw-DGE queues: wq and wk first so
        # the first projection chains can start, then the xT chunks
        load_w("wq", wq_d, nc.sync)
        load_w("wk", wk_d, nc.scalar)
        nc.gpsimd.dma_start(bq_sb[:], bq_d[:, :])
        nc.gpsimd.dma_start(bk_sb[:], bk_d[:, :])
        load_w("wo", wo_d, nc.gpsimd)

        # x^T loads: 512-column chunks, chunk-major, split across the
        # sync and scalar hw DMA queues
        xTp = st.enter_context(tc.tile_pool(name="xT", bufs=1))
        xT = [xTp.tile([128, S], BF16, name=f"xT{s}", tag=f"xT{s}")
              for s in range(8)]
        for c in range(4):
            for s in range(8):
                eng = nc.sync if s < 4 else nc.scalar
                eng.dma_start(xT[s][:, ds(512 * c, 512)],
                              xT_d.ap()[ds(128 * s, 128), ds(512 * c, 512)])
            if c == 0:
                load_w("wv", wv_d, nc.sync)

        qkv = st.enter_context(tc.tile_pool(name="qkv", bufs=1))
        qT = [qkv.tile([128, S], BF16, name=f"qT{j}", tag=f"qT{j}")
              for j in range(2)]
        kT = [qkv.tile([128, S], BF16, name=f"kT{j}", tag=f"kT{j}")
              for j in range(2)]
        ctxT = [qkv.tile([128, S], BF16, name=f"cT{j}", tag=f"cT{j}")
                for j in range(2)]
        # V padded per head with a ones column: head h at cols 65h..65h+63
        v_sb = [qkv.tile([128, 260], BF16, name=f"v{i}", tag=f"v{i}")
                for i in range(16)]

        dram = st.enter_context(tc.tile_pool(name="dram", bufs=1, space="DRAM"))
        partial_c = [dram.tile([512, H], BF16, name=f"pc{i}", tag=f"pc{i}")
                     for i in range(4)]
        red_c = [dram.tile([512 if USE_A2A else 128, H], BF16,
                           name=f"rc{i}", tag=f"rc{i}")
                 for i in range(4)]

        with tc.tile_pool(name="scps", bufs=2, space="PSUM") as scps, \
             tc.tile_pool(name="ctxps", bufs=2, space="PSUM") as ctxps, \
             tc.tile_pool(name="fillps", bufs=2, space="PSUM") as fillps, \
             tc.tile_pool(name="psb", bufs=4) as psb, \
             tc.tile_pool(name="nrm", bufs=4) as nrm, \
             tc.tile_pool(name="osb", bufs=3) as osb, \
             tc.tile_pool(name="red", bufs=2) as red:

            # ---- filler chains (run interleaved with attention) ----
            # Consecutive matmuls into the SAME PSUM bank serialize
            # (~375ns/mm); alternating banks dual-pipeline (~213ns/mm).
            # So each filler emits TWO independent accumulation chains
            # with their matmuls interleaved across two banks.
            def qk_pair(which, c):
                w = w_sb["wq"] if which == "q" else w_sb["wk"]
                dstT = qT if which == "q" else kT
                bias = bq_sb if which == "q" else bk_sb

                def emit():
                    psA = fillps.tile([128, 512], F32, tag="fill")
                    psB = fillps.tile([128, 512], F32, tag="fill")
                    for s in range(8):
                        mm(psA[:], w[:, ds(256 * s, 128)],
                           xT[s][:, ds(512 * c, 512)], s == 0, s == 7)
                        mm(psB[:], w[:, ds(256 * s + 128, 128)],
                           xT[s][:, ds(512 * c, 512)], s == 0, s == 7)
                    nc.vector.tensor_scalar_add(
                        dstT[0][:, ds(512 * c, 512)], psA[:], bias[:, ds(0, 1)])
                    nc.vector.tensor_scalar_add(
                        dstT[1][:, ds(512 * c, 512)], psB[:], bias[:, ds(1, 1)])
                return emit

            def v_pair(tv0, tv1):
                def emit():
                    psA = fillps.tile([128, 512], F32, tag="fill")
                    psB = fillps.tile([128, 512], F32, tag="fill")
                    for s in range(8):
                        mm(psA[:, 0:256], xT[s][:, ds(128 * tv0, 128)],
                           w_sb["wv"][:, ds(256 * s, 256)], s == 0, s == 7)
                        if tv1 is not None:
                            mm(psB[:, 0:256], xT[s][:, ds(128 * tv1, 128)],
                               w_sb["wv"][:, ds(256 * s, 256)], s == 0, s == 7)
                    for tv, ps in ((tv0, psA), (tv1, psB)):
                        if tv is None:
                            continue
                        nc.vector.memset(v_sb[tv][:], 1.0)
                        nc.vector.tensor_copy(
                            v_sb[tv][:].rearrange("p (h c) -> p h c", c=65)[:, :, 0:64],
                            ps[:, 0:256].rearrange("p (h c) -> p h c", c=64))
                return emit

            stages = {}

            def o_pair(tq, tl):
                def emit():
                    tt = 4 * tq + tl
                    if tl == 0:
                        stages[tq] = osb.tile([128, 4096], BF16,
                                              name=f"ot{tq}", tag="ot")
                    stage = stages[tq]
                    psA = fillps.tile([128, 512], F32, tag="fill")
                    psB = fillps.tile([128, 512], F32, tag="fill")
                    for idx, js in enumerate((1, 0)):
                        mm(psA[:], ctxT[js][:, ds(128 * tt, 128)],
                           w_sb["wo"][:, ds(1024 * js, 512)],
                           idx == 0, idx == 1)
                        mm(psB[:], ctxT[js][:, ds(128 * tt, 128)],
                           w_sb["wo"][:, ds(1024 * js + 512, 512)],
                           idx == 0, idx == 1)
                    nc.vector.tensor_copy(stage[:, ds(1024 * tl, 512)], psA[:])
                    nc.vector.tensor_copy(
                        stage[:, ds(1024 * tl + 512, 512)], psB[:])
                    # one 256KB DMA per token-block: four parallel rings
                    nc.sync.dma_start(partial_c[tq][ds(128 * tl, 128), :],
                                      stage[:, ds(1024 * tl, 1024)])
                return emit

            def red_op(tq):
                def emit():
                    if USE_A2A:
                        nc.gpsimd.collective_compute(
                            "AllToAll", mybir.AluOpType.bypass,
                            replica_groups=GROUPS,
                            ins=[partial_c[tq].opt()], outs=[red_c[tq].opt()])
                        # gather the 4 partial pieces and sum on DVE
                        gat = red.tile([128, 4096], BF16, name=f"ga{tq}",
                                       tag="gat")
                        for pc in range(4):
                            nc.sync.dma_start(
                                gat[:, ds(1024 * pc, 1024)],
                                red_c[tq][ds(128 * pc, 128), :])
                        g = gat[:].rearrange("p (pc o) -> p pc o", o=1024)
                        s01 = red.tile([128, 1024], BF16, name=f"s0{tq}",
                                       tag="s01")
                        nc.vector.tensor_add(s01[:], g[:, 0], g[:, 1])
                        s23 = red.tile([128, 1024], BF16, name=f"s2{tq}",
                                       tag="s23")
                        nc.vector.tensor_add(s23[:], g[:, 2], g[:, 3])
                        stot = red.tile([128, 1024], BF16, name=f"st{tq}",
                                        tag="stot")
                        nc.vector.tensor_add(stot[:], s01[:], s23[:])
                        for half in range(2):
                            nc.sync.dma_start(
                                out_d[ds(128 * tq, 128), ds(512 * half, 512)],
                                stot[:, ds(512 * half, 512)])
                    else:
                        nc.gpsimd.collective_compute(
                            "ReduceScatter", mybir.AluOpType.add,
                            replica_groups=GROUPS,
                            ins=[partial_c[tq].opt()], outs=[red_c[tq].opt()])
                        for half in range(2):
                            nc.gpsimd.dma_start(
                                out_d[ds(128 * tq, 128), ds(512 * half, 512)],
                                red_c[tq][:, ds(512 * half, 512)])
                return emit

            # warm up the PE while the xT/weight DMAs stream: a cold
            # tensor engine runs at less than half rate for its first
            # ~3us, so burn that ramp on junk matmuls during the loads
            junk = consts.tile([128, 512], BF16, name="junk", tag="junk")
            nc.vector.memset(junk[:], 0.0)
            junko = consts.tile([128, 8], F32, name="junko", tag="junko")
            wps = [scps.tile([128, 1024], F32, name=f"wp{i}", tag="sp")
                   for i in range(2)]
            for i in range(14):
                mm(wps[i % 2][:, 0:512], junk[:, 0:128], junk[:], True, True)
            for i in range(2):
                nc.vector.tensor_copy(junko[:, ds(4 * i, 4)], wps[i][:, 0:4])

            # pre-loop: Q(tq0), K(c0) emitted directly; V(0) pops at kt1,
            # just ahead of the (deferred) first ctx pair, so the first
            # scores/exp fire as soon as Q and K land
            qk_pair("q", 0)()
            qk_pair("k", 0)()

            fillers = deque()
            fillers.append(v_pair(0, None))
            fillers.append(v_pair(1, 2))
            fillers.append(qk_pair("k", 1))
            fillers.append(v_pair(3, 4))
            fillers.append(qk_pair("k", 2))
            fillers.append(v_pair(5, 6))
            fillers.append(qk_pair("k", 3))
            fillers.append(v_pair(7, 8))
            fillers.append(v_pair(9, 10))
            fillers.append(v_pair(11, 12))
            fillers.append(v_pair(13, 14))
            fillers.append(v_pair(15, None))

            def emit_norm(pending):
                # previous loop's normalize: broadcast raw sums via K=1
                # matmuls, wide reciprocal + multiply on DVE
                ptq, php, pcA, pcB, sms = pending
                bcs = []
                for sm16 in sms:
                    bc = fillps.tile([128, 512], F32, tag="fill")
                    mm(bc[0:64, :], ones1[:], sm16[:], True, True)
                    bcs.append(bc)
                for h, cps, bc in ((2 * php, pcA, bcs[0]),
                                   (2 * php + 1, pcB, bcs[1])):
                    rbc = nrm.tile([64, 512], F32, tag="rbc")
                    nc.vector.reciprocal_approx_fast(rbc[:], bc[0:64, :])
                    nc.vector.tensor_mul(
                        ctxT[php][ds(64 * (h % 2), 64), ds(512 * ptq, 512)],
                        cps[0:64, :], rbc[:])

            pending = None
            for li, (tq, hp) in enumerate(
                    (t, h) for t in range(4) for h in (0, 1)):
                cA = ctxps.tile([65, 512], F32, tag="cps")
                cB = ctxps.tile([65, 512], F32, tag="cps")

                def ctx_pair(kt, pt, hp=hp, cA=cA, cB=cB):
                    mm(cA[:], v_sb[kt][:, ds(65 * (2 * hp), 65)],
                       pt[:, 0:512], kt == 0, kt == 15)
                    mm(cB[:], v_sb[kt][:, ds(65 * (2 * hp + 1), 65)],
                       pt[:, 512:1024], kt == 0, kt == 15)

                # ctx pairs run one iteration deferred: a filler pop never
                # separates scores(kt+1) from exp(kt+1), so the ACT stream
                # stays gapless across pops and loop transitions
                pend_ctx = None
                for kt in range(16):
                    sp = scps.tile([128, 1024], F32, tag="sp")
                    mm(sp[:, 0:512],
                       kT[hp][0:64, ds(128 * kt, 128)],
                       qT[hp][0:64, ds(512 * tq, 512)],
                       True, True, tile_position=(0, 0))
                    mm(sp[:, 512:1024],
                       kT[hp][64:128, ds(128 * kt, 128)],
                       qT[hp][64:128, ds(512 * tq, 512)],
                       True, True, tile_position=(64, 0))
                    pt = psb.tile([128, 1024], BF16, tag="pt")
                    nc.scalar.activation(pt[:], sp[:], AF.Exp, scale=0.125)
                    if kt == 1 and pending is not None:
                        emit_norm(pending)
                        pending = None
                    # loop0 must pop from kt1 (v_sb deadlines); later loops
                    # wait until kt4 so the transition settles first
                    if kt >= (1 if li == 0 else 4) and fillers:
                        fillers.popleft()()
                    if pend_ctx is not None:
                        ctx_pair(*pend_ctx)
                    pend_ctx = (kt, pt)
                ctx_pair(*pend_ctx)
                # cast both heads' sums rows now (DVE); the bc matmuls and
                # multiplies run early in the next loop
                sms = []
                for cps in (cA, cB):
                    sm16 = nrm.tile([1, 512], BF16, tag="sm")
                    nc.vector.tensor_copy(sm16[:], cps[ds(64, 1), :])
                    sms.append(sm16)
                pending = (tq, hp, cA, cB, sms)
                if hp == 0 and tq < 3:
                    # Q projection for the next chunk pops during the
                    # otherwise-filler-free (tq, hp=1) loop, keeping the
                    # lumpy 3.4us chain pairs out of the o_pair loops
                    fillers.append(qk_pair("q", tq + 1))
                if hp == 1:
                    for tl in range(4):
                        fillers.append(o_pair(tq, tl))
                    fillers.append(red_op(tq))

            emit_norm(pending)
            while fillers:
                fillers.popleft()()

    nc.compile()
    return nc


def _get_nc():
    if "nc" not in _cache:
        _cache["nc"] = _build()
    return _cache["nc"]


def _arr_w(Wslice):
    # [1024, 256] -> [128, 2048] with w[p, 256s+j] = W[128s+p, j]
    return np.ascontiguousarray(
        Wslice.reshape(8, 128, 256).transpose(1, 0, 2).reshape(128, 2048))


def _in_maps(x, Wq, bq, Wk, bk, Wv, bv, Wo, bo):
    bf = ml_dtypes.bfloat16
    maps = []
    for c in range(NCORES):
        b, g = c // 4, c % 4
        j0 = JG * g
        wo_slice = Wo[j0:j0 + JG, :]  # [256, 1024]
        wo_arr = np.ascontiguousarray(
            wo_slice.reshape(2, 128, 1024).transpose(1, 0, 2).reshape(128, 2048))
        maps.append({
            "xT": np.ascontiguousarray(x[b].T).astype(bf),
            "wq": _arr_w(Wq[:, j0:j0 + JG]).astype(bf),
            "wk": _arr_w(Wk[:, j0:j0 + JG]).astype(bf),
            "wv": _arr_w(Wv[:, j0:j0 + JG]).astype(bf),
            "wo": wo_arr.astype(bf),
            "bqc": np.ascontiguousarray(bq[j0:j0 + JG].reshape(2, 128).T),
            "bkc": np.ascontiguousarray(bk[j0:j0 + JG].reshape(2, 128).T),
        })
    return maps


def kernel(x, Wq, bq, Wk, bk, Wv, bv, Wo, bo, _trace=False):
    x, Wq, bq, Wk, bk, Wv, bv, Wo, bo = (
        np.asarray(a, dtype=np.float32)
        for a in (x, Wq, bq, Wk, bk, Wv, bv, Wo, bo))
    nc = _get_nc()
    res = run_bass_kernel_spmd(nc, _in_maps(x, Wq, bq, Wk, bk, Wv, bv, Wo, bo),
                               core_ids=list(range(NCORES)), trace=_trace)
    out = np.empty((B, S, H), np.float32)
    for c in range(NCORES):
        b, g = c // 4, c % 4
        oc = np.asarray(res.results[c]["out"], dtype=np.float32)
        for tq in range(4):
            out[b, 512 * tq + 128 * g:512 * tq + 128 * (g + 1), :] = \
                oc[128 * tq:128 * (tq + 1)]
    out += bv @ Wo + bo  # exact: softmax rows sum to 1
    if _trace:
        return out, res
    return out


# revision 30
# speedup vs baseline: 1.1123x; 1.0446x over previous
"""Distributed multi-head attention kernel for 8 TRN2 NeuronCores.

Problem: B=2, S=2048, H=1024 (16 heads x 64), fp32 in/out.
Sharding: core c = 4*b + g handles batch b and head-group g (4 heads, 256
hidden cols). Wq/Wk/Wv column-sharded, Wo row-sharded; the cross-core
sum of output partials runs as a per-chunk collective over each 4-core
batch group.

v6: fully software-pipelined emission. Host pre-transposes x and
pre-arranges the weight tiles so every load is a contiguous 2KB-row
DMA. K/V/Q/output projections run as "filler" chain-pairs interleaved
into the attention inner loop (two accumulation chains with matmuls
alternating across two PSUM banks dual-pipeline at ~2x the rate of a
single chain). Each ctx pair is deferred one iteration so a filler pop
never sits between scores and the next exp. The exp() stream on ACT
(~136us) and the PE stream (~140us) are co-critical.

Dataflow per core (transpose-free attention, bf16 matmuls, fp32 PSUM):
  Q^T,K^T = (W^T x^T) in [j,t] layout; V = x^T-stationary @ Wv
  scores^T[k,q] = K^T.T@Q^T, two heads packed into PE row halves (K=64)
  Pt = exp(scores/8) (scores ~ N(0,1): exact softmax, no max pass)
  ctx^T[d,q] (+ sums row via ones column in V) = [V|1].T @ Pt
  normalize via K=1 broadcast matmul of sums + wide reciprocal (DVE)
  out partial[t,o] = ctx^T-stationary @ Wo -> bf16 -> per-chunk
  AllToAll + local DVE adds (or ReduceScatter with USE_A2A=False).
bq/bk applied on-device (DVE bias-add); bv/bo folded on host (exact:
out += bv@Wo + bo, since softmax rows sum to one).
"""

import sys

for p in ("/opt/trn_rl_repo",):
    if p not in sys.path:
        sys.path.insert(0, p)

from collections import deque
from contextlib import ExitStack

import ml_dtypes
import numpy as np

from concourse import bacc, mybir, tile
from concourse.bass import ds
from concourse.bass_utils import run_bass_kernel_spmd

F32 = mybir.dt.float32
BF16 = mybir.dt.bfloat16
AF = mybir.ActivationFunctionType

B, S, H = 2, 2048, 1024
NH, D = 16, 64
NCORES = 8
GROUPS = [[0, 1, 2, 3], [4, 5, 6, 7]]
JG = 256           # hidden cols per core (4 heads)
SO = S // 4        # 512 output rows per core after the collective

USE_A2A = False    # AllToAll needs >4-core groups; RS it is

_cache = {}


def _build():
    nc = bacc.Bacc("TRN2", target_bir_lowering=False, debug=False,
                   num_devices=NCORES)
    # weights arrive pre-arranged on host: w*[p, inner*s + j] = W[128s+p, j]
    xT_d = nc.dram_tensor("xT", [H, S], BF16, kind="ExternalInput")
    wq_d = nc.dram_tensor("wq", [128, 2048], BF16, kind="ExternalInput")
    wk_d = nc.dram_tensor("wk", [128, 2048], BF16, kind="ExternalInput")
    wv_d = nc.dram_tensor("wv", [128, 2048], BF16, kind="ExternalInput")
    wo_d = nc.dram_tensor("wo", [128, 2048], BF16, kind="ExternalInput")
    bq_d = nc.dram_tensor("bqc", [128, 2], F32, kind="ExternalInput")
    bk_d = nc.dram_tensor("bkc", [128, 2], F32, kind="ExternalInput")
    out_d = nc.dram_tensor("out", [SO, H], BF16, kind="ExternalOutput")

    def mm(ps, lhsT, rhs, start, stop, tile_position=None):
        nc.tensor.matmul(ps, lhsT, rhs, start=start, stop=stop,
                         tile_position=tile_position)

    with tile.TileContext(nc) as tc, ExitStack() as st:
        consts = st.enter_context(tc.tile_pool(name="consts", bufs=1))
        ones1 = consts.tile([1, 64], BF16)
        nc.vector.memset(ones1[:], 1.0)
        bq_sb = consts.tile([128, 2], F32)
        bk_sb = consts.tile([128, 2], F32)

        wpool = st.enter_context(tc.tile_pool(name="weights", bufs=1))
        w_sb = {}

        def load_w(wname, wd, eng):
            wt = wpool.tile([128, 2048], BF16, name=f"{wname}sb",
                            tag=f"{wname}sb")
            eng.dma_start(wt[:], wd.ap()[:, :])
            w_sb[wname] = wt

        # deadline order, on the fast hw-DGE queues: wq and wk first so
        # the first projection chains can start, then the xT chunks
        load_w("wq", wq_d, nc.sync)
        load_w("wk", wk_d, nc.scalar)
        nc.gpsimd.dma_start(bq_sb[:], bq_d[:, :])
        nc.gpsimd.dma_start(bk_sb[:], bk_d[:, :])
        load_w("wo", wo_d, nc.gpsimd)

        # x^T loads: 512-column chunks, chunk-major, split across the
        # sync and scalar hw DMA queues
        xTp = st.enter_context(tc.tile_pool(name="xT", bufs=1))
        xT = [xTp.tile([128, S], BF16, name=f"xT{s}", tag=f"xT{s}")
              for s in range(8)]
        # only chunk 0 uses the scalar queue: later issues there would
        # queue ahead of the first exp() and delay the whole ACT stream
        for c in range(4):
            for s in range(8):
                eng = nc.scalar if (c == 0 and s >= 4) else nc.sync
                eng.dma_start(xT[s][:, ds(512 * c, 512)],
                              xT_d.ap()[ds(128 * s, 128), ds(512 * c, 512)])
            if c == 0:
                load_w("wv", wv_d, nc.sync)

        qkv = st.enter_context(tc.tile_pool(name="qkv", bufs=1))
        qT = [qkv.tile([128, S], BF16, name=f"qT{j}", tag=f"qT{j}")
              for j in range(2)]
        kT = [qkv.tile([128, S], BF16, name=f"kT{j}", tag=f"kT{j}")
              for j in range(2)]
        ctxT = [qkv.tile([128, S], BF16, name=f"cT{j}", tag=f"cT{j}")
                for j in range(2)]
        # V padded per head with a ones column: head h at cols 65h..65h+63
        v_sb = [qkv.tile([128, 260], BF16, name=f"v{i}", tag=f"v{i}")
                for i in range(16)]

        dram = st.enter_context(tc.tile_pool(name="dram", bufs=1, space="DRAM"))
        partial_c = [dram.tile([512, H], BF16, name=f"pc{i}", tag=f"pc{i}")
                     for i in range(4)]
        red_c = [dram.tile([512 if USE_A2A else 128, H], BF16,
                           name=f"rc{i}", tag=f"rc{i}")
                 for i in range(4)]

        with tc.tile_pool(name="scps", bufs=2, space="PSUM") as scps, \
             tc.tile_pool(name="ctxps", bufs=2, space="PSUM") as ctxps, \
             tc.tile_pool(name="fillps", bufs=2, space="PSUM") as fillps, \
             tc.tile_pool(name="psb", bufs=4) as psb, \
             tc.tile_pool(name="nrm", bufs=4) as nrm, \
             tc.tile_pool(name="osb", bufs=3) as osb, \
             tc.tile_pool(name="red", bufs=2) as red:

            # ---- filler chains (run interleaved with attention) ----
            # Consecutive matmuls into the SAME PSUM bank serialize
            # (~375ns/mm); alternating banks dual-pipeline (~213ns/mm).
            # So each filler emits TWO independent accumulation chains
            # with their matmuls interleaved across two banks.
            def qk_pair(which, c):
                w = w_sb["wq"] if which == "q" else w_sb["wk"]
                dstT = qT if which == "q" else kT
                bias = bq_sb if which == "q" else bk_sb

                def emit():
                    psA = fillps.tile([128, 512], F32, tag="fill")
                    psB = fillps.tile([128, 512], F32, tag="fill")
                    for s in range(8):
                        mm(psA[:], w[:, ds(256 * s, 128)],
                           xT[s][:, ds(512 * c, 512)], s == 0, s == 7)
                        mm(psB[:], w[:, ds(256 * s + 128, 128)],
                           xT[s][:, ds(512 * c, 512)], s == 0, s == 7)
                    nc.vector.tensor_scalar_add(
                        dstT[0][:, ds(512 * c, 512)], psA[:], bias[:, ds(0, 1)])
                    nc.vector.tensor_scalar_add(
                        dstT[1][:, ds(512 * c, 512)], psB[:], bias[:, ds(1, 1)])
                return emit

            def v_pair(tv0, tv1):
                def emit():
                    psA = fillps.tile([128, 512], F32, tag="fill")
                    psB = fillps.tile([128, 512], F32, tag="fill")
                    for s in range(8):
                        mm(psA[:, 0:256], xT[s][:, ds(128 * tv0, 128)],
                           w_sb["wv"][:, ds(256 * s, 256)], s == 0, s == 7)
                        if tv1 is not None:
                            mm(psB[:, 0:256], xT[s][:, ds(128 * tv1, 128)],
                               w_sb["wv"][:, ds(256 * s, 256)], s == 0, s == 7)
                    for tv, ps in ((tv0, psA), (tv1, psB)):
                        if tv is None:
                            continue
                        nc.vector.memset(v_sb[tv][:], 1.0)
                        nc.vector.tensor_copy(
                            v_sb[tv][:].rearrange("p (h c) -> p h c", c=65)[:, :, 0:64],
                            ps[:, 0:256].rearrange("p (h c) -> p h c", c=64))
                return emit

            stages = {}
            outs_emit = []

            def o_pair(tq, tl):
                def emit():
                    tt = 4 * tq + tl
                    if tl == 0:
                        stages[tq] = osb.tile([128, 4096], BF16,
                                              name=f"ot{tq}", tag="ot")
                    stage = stages[tq]
                    psA = fillps.tile([128, 512], F32, tag="fill")
                    psB = fillps.tile([128, 512], F32, tag="fill")
                    for idx, js in enumerate((1, 0)):
                        mm(psA[:], ctxT[js][:, ds(128 * tt, 128)],
                           w_sb["wo"][:, ds(1024 * js, 512)],
                           idx == 0, idx == 1)
                        mm(psB[:], ctxT[js][:, ds(128 * tt, 128)],
                           w_sb["wo"][:, ds(1024 * js + 512, 512)],
                           idx == 0, idx == 1)
                    nc.vector.tensor_copy(stage[:, ds(1024 * tl, 512)], psA[:])
                    nc.vector.tensor_copy(
                        stage[:, ds(1024 * tl + 512, 512)], psB[:])
                    # one 256KB DMA per token-block: four parallel rings
                    nc.sync.dma_start(partial_c[tq][ds(128 * tl, 128), :],
                                      stage[:, ds(1024 * tl, 1024)])
                return emit

            def red_op(tq):
                def emit():
                    if USE_A2A:
                        nc.gpsimd.collective_compute(
                            "AllToAll", mybir.AluOpType.bypass,
                            replica_groups=GROUPS,
                            ins=[partial_c[tq].opt()], outs=[red_c[tq].opt()])
                        # gather the 4 partial pieces and sum on DVE
                        gat = red.tile([128, 4096], BF16, name=f"ga{tq}",
                                       tag="gat")
                        for pc in range(4):
                            nc.sync.dma_start(
                                gat[:, ds(1024 * pc, 1024)],
                                red_c[tq][ds(128 * pc, 128), :])
                        g = gat[:].rearrange("p (pc o) -> p pc o", o=1024)
                        s01 = red.tile([128, 1024], BF16, name=f"s0{tq}",
                                       tag="s01")
                        nc.vector.tensor_add(s01[:], g[:, 0], g[:, 1])
                        s23 = red.tile([128, 1024], BF16, name=f"s2{tq}",
                                       tag="s23")
                        nc.vector.tensor_add(s23[:], g[:, 2], g[:, 3])
                        stot = red.tile([128, 1024], BF16, name=f"st{tq}",
                                        tag="stot")
                        nc.vector.tensor_add(stot[:], s01[:], s23[:])
                        for half in range(2):
                            nc.sync.dma_start(
                                out_d[ds(128 * tq, 128), ds(512 * half, 512)],
                                stot[:, ds(512 * half, 512)])
                    else:
                        nc.gpsimd.collective_compute(
                            "ReduceScatter", mybir.AluOpType.add,
                            replica_groups=GROUPS,
                            ins=[partial_c[tq].opt()], outs=[red_c[tq].opt()])

                        # the out DMA waits on its RS: deferring it past
                        # all RS triggers keeps the gpsimd queue from
                        # stalling the next trigger behind this wait
                        def out_dma():
                            for half in range(2):
                                nc.gpsimd.dma_start(
                                    out_d[ds(128 * tq, 128),
                                          ds(512 * half, 512)],
                                    red_c[tq][:, ds(512 * half, 512)])
                        outs_emit.append(out_dma)
                return emit

            # warm up the PE while the xT/weight DMAs stream: a cold
            # tensor engine runs at less than half rate for its first
            # ~3us, so burn that ramp on junk matmuls during the loads
            junk = consts.tile([128, 512], BF16, name="junk", tag="junk")
            nc.vector.memset(junk[:], 0.0)
            junko = consts.tile([128, 8], F32, name="junko", tag="junko")
            wps = [scps.tile([128, 1024], F32, name=f"wp{i}", tag="sp")
                   for i in range(2)]
            for i in range(14):
                mm(wps[i % 2][:, 0:512], junk[:, 0:128], junk[:], True, True)
            for i in range(2):
                nc.vector.tensor_copy(junko[:, ds(4 * i, 4)], wps[i][:, 0:4])

            # pre-loop: Q(tq0), K(c0), V(0) emitted directly; the K-chain
            # fillers lead the V ones so scores(kt) never wait on kT
            qk_pair("q", 0)()
            qk_pair("k", 0)()
            v_pair(0, None)()

            fillers = deque()
            fillers.append(qk_pair("k", 1))
            fillers.append(v_pair(1, 2))
            fillers.append(qk_pair("k", 2))
            fillers.append(v_pair(3, 4))
            fillers.append(v_pair(5, 6))
            fillers.append(qk_pair("k", 3))
            fillers.append(v_pair(7, 8))
            fillers.append(v_pair(9, 10))
            fillers.append(v_pair(11, 12))
            fillers.append(v_pair(13, 14))
            fillers.append(v_pair(15, None))

            def emit_norm(pending):
                # previous loop's normalize: broadcast raw sums via K=1
                # matmuls, wide reciprocal + multiply on DVE
                ptq, php, pcA, pcB, sms = pending
                bcs = []
                for sm16 in sms:
                    bc = fillps.tile([128, 512], F32, tag="fill")
                    mm(bc[0:64, :], ones1[:], sm16[:], True, True)
                    bcs.append(bc)
                for h, cps, bc in ((2 * php, pcA, bcs[0]),
                                   (2 * php + 1, pcB, bcs[1])):
                    rbc = nrm.tile([64, 512], F32, tag="rbc")
                    nc.vector.reciprocal_approx_fast(rbc[:], bc[0:64, :])
                    nc.vector.tensor_mul(
                        ctxT[php][ds(64 * (h % 2), 64), ds(512 * ptq, 512)],
                        cps[0:64, :], rbc[:])

            pending = None
            for li, (tq, hp) in enumerate(
                    (t, h) for t in range(4) for h in (0, 1)):
                cA = ctxps.tile([65, 512], F32, tag="cps")
                cB = ctxps.tile([65, 512], F32, tag="cps")

                def ctx_pair(kt, pt, hp=hp, cA=cA, cB=cB):
                    mm(cA[:], v_sb[kt][:, ds(65 * (2 * hp), 65)],
                       pt[:, 0:512], kt == 0, kt == 15)
                    mm(cB[:], v_sb[kt][:, ds(65 * (2 * hp + 1), 65)],
                       pt[:, 512:1024], kt == 0, kt == 15)

                # ctx pairs run one iteration deferred: a filler pop never
                # separates scores(kt+1) from exp(kt+1), so the ACT stream
                # stays gapless across pops and loop transitions
                pend_ctx = None
                for kt in range(16):
                    sp = scps.tile([128, 1024], F32, tag="sp")
                    mm(sp[:, 0:512],
                       kT[hp][0:64, ds(128 * kt, 128)],
                       qT[hp][0:64, ds(512 * tq, 512)],
                       True, True, tile_position=(0, 0))
                    mm(sp[:, 512:1024],
                       kT[hp][64:128, ds(128 * kt, 128)],
                       qT[hp][64:128, ds(512 * tq, 512)],
                       True, True, tile_position=(64, 0))
                    pt = psb.tile([128, 1024], BF16, tag="pt")
                    nc.scalar.activation(pt[:], sp[:], AF.Exp, scale=0.125)
                    if kt == 1 and pending is not None:
                        emit_norm(pending)
                        pending = None
                    # loop0 must pop from kt1 (v_sb deadlines); later loops
                    # wait until kt4 so the transition settles first
                    if kt >= (1 if li == 0 else 4) and fillers:
                        fillers.popleft()()
                    if pend_ctx is not None:
                        ctx_pair(*pend_ctx)
                    pend_ctx = (kt, pt)
                ctx_pair(*pend_ctx)
                # cast both heads' sums rows now (DVE); the bc matmuls and
                # multiplies run early in the next loop
                sms = []
                for cps in (cA, cB):
                    sm16 = nrm.tile([1, 512], BF16, tag="sm")
                    nc.vector.tensor_copy(sm16[:], cps[ds(64, 1), :])
                    sms.append(sm16)
                pending = (tq, hp, cA, cB, sms)
                if hp == 0 and tq < 3:
                    # Q projection for the next chunk pops during the
                    # otherwise-filler-free (tq, hp=1) loop, keeping the
                    # lumpy 3.4us chain pairs out of the o_pair loops
                    fillers.append(qk_pair("q", tq + 1))
                if hp == 1:
                    for tl in range(4):
                        fillers.append(o_pair(tq, tl))
                    fillers.append(red_op(tq))

            emit_norm(pending)
            while fillers:
                fillers.popleft()()
            for f in outs_emit:
                f()

    nc.compile()
    return nc


def _get_nc():
    if "nc" not in _cache:
        _cache["nc"] = _build()
    return _cache["nc"]


def _arr_w(Wslice):
    # [1024, 256] -> [128, 2048] with w[p, 256s+j] = W[128s+p, j]
    return np.ascontiguousarray(
        Wslice.reshape(8, 128, 256).transpose(1, 0, 2).reshape(128, 2048))


def _in_maps(x, Wq, bq, Wk, bk, Wv, bv, Wo, bo):
    bf = ml_dtypes.bfloat16
    maps = []
    for c in range(NCORES):
        b, g = c // 4, c % 4
        j0 = JG * g
        wo_slice = Wo[j0:j0 + JG, :]  # [256, 1024]
        wo_arr = np.ascontiguousarray(
            wo_slice.reshape(2, 128, 1024).transpose(1, 0, 2).reshape(128, 2048))
        maps.append({
            "xT": np.ascontiguousarray(x[b].T).astype(bf),
            "wq": _arr_w(Wq[:, j0:j0 + JG]).astype(bf),
            "wk": _arr_w(Wk[:, j0:j0 + JG]).astype(bf),
            "wv": _arr_w(Wv[:, j0:j0 + JG]).astype(bf),
            "wo": wo_arr.astype(bf),
            "bqc": np.ascontiguousarray(bq[j0:j0 + JG].reshape(2, 128).T),
            "bkc": np.ascontiguousarray(bk[j0:j0 + JG].reshape(2, 128).T),
        })
    return maps


def kernel(x, Wq, bq, Wk, bk, Wv, bv, Wo, bo, _trace=False):
    x, Wq, bq, Wk, bk, Wv, bv, Wo, bo = (
        np.asarray(a, dtype=np.float32)
        for a in (x, Wq, bq, Wk, bk, Wv, bv, Wo, bo))
    nc = _get_nc()
    res = run_bass_kernel_spmd(nc, _in_maps(x, Wq, bq, Wk, bk, Wv, bv, Wo, bo),
                               core_ids=list(range(NCORES)), trace=_trace)
    out = np.empty((B, S, H), np.float32)
    for c in range(NCORES):
        b, g = c // 4, c % 4
        oc = np.asarray(res.results[c]["out"], dtype=np.float32)
        for tq in range(4):
            out[b, 512 * tq + 128 * g:512 * tq + 128 * (g + 1), :] = \
                oc[128 * tq:128 * (tq + 1)]
    out += bv @ Wo + bo  # exact: softmax rows sum to 1
    if _trace:
        return out, res
    return out
